# revision 5
# baseline (speedup 1.0000x reference)
"""GCGRUCell Trainium2 kernel — 8-core SPMD (v3: fp8 gather path, piecewise
AG/BC pipelining, consolidated gathers).

Math (per reference):
  value = sigmoid(cat([x, h]) @ W_fc + b_fc);  r, u = split(value)
  X0 = cat([x, r*h])                                (B, N, D)   D=66
  Y  = X0 @ Wg_odd                                  (B, N, U)
  X1 = S @ Y          (segment_sum over E edges, commuted with Wg_odd)
  c  = tanh(X0 @ Wg_even + X1 + b_g)
  out = u * hx + (1-u) * c

Sharding: nodes 1250/core (phase A local, all batches); Y AllGather to a
(10240, 1024) fp8e4 table in 5 pieces; phase B edges are grouped by
(dest block, source AG piece) and processed piece-major with one gather
per (piece, half-of-blocks) — the gather for piece p waits only on AG
piece p, so phase B pipelines with the AllGather. Per-block
one-hot-vals fp8 matmuls on PE (fp32 PSUM) accumulate into an SBUF x1
accumulator (z0 folded in at piece 0); phase C combines and writes the
core's output slice.

Row order: tile t in [0,160) = (blk = t//16, b = t%16), 128 nodes per
tile. Feature order: [h (0:64), xi (64:66), ones (66)] (32-aligned
partition bases). Biases ride the ones row.
"""

from contextlib import nullcontext

import numpy as np
import concourse.bass as bass
import concourse.bacc as bacc
import concourse.mybir as mybir
import concourse.tile as tile
from bass_rust import add_dep_helper
from concourse.bass_utils import run_bass_kernel_spmd

F32 = mybir.dt.float32
BF16 = mybir.dt.bfloat16
F8 = mybir.dt.float8e4
I16 = mybir.dt.int16

NCORES = 8
N, B, U, DIN = 10000, 16, 64, 2
D = DIN + U                      # 66
K = D + 1                        # 67 (ones row for biases)
NPC = N // NCORES                # 1250 nodes/core
NPC_PAD = 1280                   # 10 blocks of 128
NBLK = NPC_PAD // 128            # 10 dest blocks
ROWS = B * NPC_PAD               # 20480
CHUNK_ROWS = 16 * 128            # one block of nodes x all batches
GPC = CHUNK_ROWS // 512          # 4 512-col groups per chunk
W = B * U                        # 1024 gather row width
AGS = 5                          # allgather split count
AG_BOUNDS = [round(i * NBLK / AGS) for i in range(AGS + 1)]
GROUPS = [(p, j) for p in range(AGS) for j in range(NBLK)]
HALVES = [(p, h) for p in range(AGS) for h in range(2)]
JPH = NBLK // 2                  # dest blocks per gather call
NQ = 1                           # DMA queues to spread gathers over


def _ytab_row(c, nl):
    """y_tab row index of node (core c, local node nl) after the split
    AllGather: piece p's output is [rank0 blocks j0:j1, rank1 blocks
    j0:j1, ...] concatenated."""
    j = nl // 128
    si = nl % 128
    bounds = np.asarray(AG_BOUNDS)
    p = np.searchsorted(bounds, j, side="right") - 1
    j0 = bounds[p]
    j1 = bounds[p + 1]
    return (j0 * NCORES + c * (j1 - j0) + (j - j0)) * 128 + si


def _src_piece(c):
    """AG piece that carries the y_tab row of global node c."""
    j = (c % NPC) // 128
    return np.searchsorted(np.asarray(AG_BOUNDS), j, side="right") - 1


def build_kernel(eb: tuple[int, ...], stage: int = 5,
                 r_a: int = 1, r_ag: int = 1, r_bc: int = 1):
    """eb[g] = padded edge count of group g = (piece-major) (p, j),
    multiple of 128 (same for all cores).
    stage: 1=phase A only, 2=+allgather, 3=+gathers, 4=full
    r_a/r_ag/r_bc: repetition counts (hardware For_i loops / replicated
    collective) for wall-clock-difference timing; correctness needs 1."""
    assert len(eb) == AGS * NBLK and all(e % 128 == 0 for e in eb)
    kt_tot = sum(e // 128 for e in eb)
    # per (piece, half): total padded edges of its JPH dest-block groups
    eh = [sum(eb[p * NBLK + h * JPH + j] for j in range(JPH))
          for (p, h) in HALVES]
    ktmax = max(e // 128 for e in eh)

    nc = bacc.Bacc("TRN2", target_bir_lowering=False, debug=False,
                   num_devices=NCORES)

    # ---- I/O ----
    catT_in = nc.dram_tensor("catT", [K, ROWS], BF16, kind="ExternalInput")
    hxb_in = nc.dram_tensor("hx_blk", [128, NBLK, W], BF16,
                            kind="ExternalInput")
    wfc_in = nc.dram_tensor("wfc", [K, 128], BF16, kind="ExternalInput")
    wg_in = nc.dram_tensor("wg", [K, 128], BF16, kind="ExternalInput")
    idx_in = nc.dram_tensor("idxw", [128, sum(eb) // 16], I16,
                            kind="ExternalInput")
    oh_in = nc.dram_tensor("oh", [128, kt_tot * 128], F8,
                           kind="ExternalInput")
    out_dram = nc.dram_tensor("out", [NBLK, 128, B, U], F32,
                              kind="ExternalOutput")

    # ---- internal DRAM (collective) ----
    y_loc = nc.dram_tensor("y_loc", [NBLK * 128, W], F8, kind="Internal")
    y_tab = nc.dram_tensor("y_tab", [NBLK * NCORES * 128, W], F8,
                           kind="Internal", addr_space="Shared")

    with tile.TileContext(nc) as tc:
        with (
            tc.tile_pool(name="persist", bufs=1) as pp,
        ):
            wfc = pp.tile([K, 128], BF16)
            nc.sync.dma_start(wfc[:], wfc_in[:])
            wg = pp.tile([K, 128], BF16)
            nc.sync.dma_start(wg[:], wg_in[:])
            z0 = pp.tile([128, NBLK * W], BF16)     # 20KB/part
            usb = pp.tile([128, NBLK * W], BF16)    # 20KB/part
            x1acc = pp.tile([128, NBLK * W], F32)   # 40KB/part
            hx_sb = pp.tile([128, NBLK, W], BF16)   # 20KB/part

            # ================= PHASE A =================
            y_writes = []
            with (
                tc.tile_pool(name="pa", bufs=2) as pa,
                tc.tile_pool(name="pa_sig", bufs=3) as psig,
                tc.tile_pool(name="pa_y", bufs=2) as py,
                tc.tile_pool(name="ps_r", bufs=2, space="PSUM") as ps_r,
                tc.tile_pool(name="ps_u", bufs=2, space="PSUM") as ps_u,
                tc.tile_pool(name="ps_yz", bufs=2, space="PSUM") as ps_yz,
                tc.For_i(0, r_a, 1) if r_a > 1 else nullcontext(),
            ):
                for blk in range(NBLK):
                    catT = pa.tile([K, CHUNK_ROWS], BF16, tag="catT")
                    nc.sync.dma_start(
                        catT[:],
                        catT_in[:, blk * CHUNK_ROWS:(blk + 1) * CHUNK_ROWS])
                    x0t = pa.tile([K, CHUNK_ROWS], BF16, tag="x0t")
                    nc.vector.tensor_copy(x0t[U:K, :], catT[U:K, :])
                    for g in range(GPC):
                        sl = slice(g * 512, (g + 1) * 512)
                        pr = ps_r.tile([U, 512], F32)
                        nc.tensor.matmul(pr[:], wfc[:, 0:U], catT[:, sl],
                                         start=True, stop=True)
                        sig = psig.tile([U, 512], BF16)
                        nc.scalar.activation(
                            sig[:], pr[:], mybir.ActivationFunctionType.Sigmoid)
                        nc.vector.tensor_mul(
                            x0t[0:U, sl], sig[:], catT[0:U, sl])
                    ystage = py.tile([128, W], F8, tag="ystage")
                    # u-logits: 8 tiles share one PSUM bank -> 1 big sigmoid
                    for half in range(2):
                        pu = ps_u.tile([128, 512], F32)
                        for i in range(8):
                            b = half * 8 + i
                            tsl = slice(b * 128, (b + 1) * 128)
                            nc.tensor.matmul(
                                pu[:, i * U:(i + 1) * U],
                                catT[:, tsl], wfc[:, U:128],
                                start=True, stop=True)
                        nc.scalar.activation(
                            usb[:, blk * W + half * 512:
                                 blk * W + (half + 1) * 512],
                            pu[:], mybir.ActivationFunctionType.Sigmoid)
                    # Y and Z0' as separate families: 8 tiles share a PSUM
                    # bank each -> 2 big contiguous copies per half
                    for half in range(2):
                        pyy = ps_yz.tile([128, 512], F32, tag="yy")
                        pzz = ps_yz.tile([128, 512], F32, tag="zz")
                        for i in range(8):
                            b = half * 8 + i
                            tsl = slice(b * 128, (b + 1) * 128)
                            nc.tensor.matmul(
                                pyy[:, i * U:(i + 1) * U],
                                x0t[:, tsl], wg[:, 0:U],
                                start=True, stop=True)
                            nc.tensor.matmul(
                                pzz[:, i * U:(i + 1) * U],
                                x0t[:, tsl], wg[:, U:128],
                                start=True, stop=True)
                        nc.vector.tensor_copy(
                            ystage[:, half * 512:(half + 1) * 512], pyy[:])
                        nc.vector.tensor_copy(
                            z0[:, blk * W + half * 512:
                               blk * W + (half + 1) * 512], pzz[:])
                    ydma = nc.sync.dma_start(
                        y_loc[blk * 128:(blk + 1) * 128, :], ystage[:])
                    y_writes.append(ydma)

            # hx prefetch for phase C (after phase A's catT loads in the
            # sync queue so it doesn't delay them)
            nc.sync.dma_start(hx_sb[:], hxb_in[:])

            # ================= ALLGATHER (split, overlaps phase A) ======
            ccs = []
            if stage >= 2:
                prev_cc = None
                for rep in range(r_ag):
                    for p in range(AGS):
                        j0, j1 = AG_BOUNDS[p], AG_BOUNDS[p + 1]
                        cc = nc.gpsimd.collective_compute(
                            "AllGather", mybir.AluOpType.bypass,
                            replica_groups=[list(range(NCORES))],
                            ins=[y_loc[j0 * 128: j1 * 128, :]],
                            outs=[y_tab[j0 * NCORES * 128:
                                        j1 * NCORES * 128, :]],
                        )
                        ccs.append(cc)
                        if r_ag > 1 and prev_cc is not None:
                            add_dep_helper(cc.ins, prev_cc.ins, sync=True,
                                           reason="serialize ag reps")
                        prev_cc = cc
                        if r_a == 1 and r_ag == 1:
                            for j in range(j0, j1):
                                add_dep_helper(cc.ins, y_writes[j].ins,
                                               sync=True,
                                               reason="allgather reads y_loc")

            # ================= PHASE B + C =================
            with (
                tc.tile_pool(name="pg", bufs=3) as pg,
                tc.tile_pool(name="pi", bufs=3) as pi,
                tc.tile_pool(name="po", bufs=2) as po,
                tc.tile_pool(name="pc", bufs=2) as pcl,
                tc.tile_pool(name="ps_b", bufs=2, space="PSUM") as ps_b,
                tc.For_i(0, r_bc, 1) if r_bc > 1 else nullcontext(),
            ):
                idx_off = 0   # in int16 columns (16 idx each)
                kt_off = 0
                for hi, (p, h) in enumerate(HALVES if stage >= 3 else []):
                    e_h = eh[hi]
                    nkt_h = e_h // 128
                    idxt = pi.tile([128, max(eh) // 16], I16, tag="idx")
                    nc.sync.dma_start(
                        idxt[:, :e_h // 16],
                        idx_in[:, idx_off: idx_off + e_h // 16])
                    gt = pg.tile([128, ktmax, W], F8, tag="G")
                    gather = nc.gpsimd.dma_gather(
                        out_ap=gt[:, :nkt_h, :],
                        in_ap=y_tab[:],
                        idxs_ap=idxt[:, :e_h // 16],
                        num_idxs=e_h,
                        num_idxs_reg=e_h,
                        elem_size=W,
                        single_packet=False,
                        queue_num=hi % NQ,
                    )
                    if r_bc == 1 and r_ag == 1:
                        add_dep_helper(
                            gather.ins, ccs[p].ins, sync=True,
                            reason="gather reads allgathered y_tab piece")
                    idx_off += e_h // 16
                    if stage < 4:
                        kt_off += nkt_h
                        continue
                    oht = po.tile([128, ktmax * 128], F8, tag="O")
                    nc.sync.dma_start(
                        oht[:, :nkt_h * 128],
                        oh_in[:, kt_off * 128:(kt_off + nkt_h) * 128])
                    ktb = 0
                    for jj in range(JPH):
                        j = h * JPH + jj
                        nkt = eb[p * NBLK + j] // 128
                        px1 = ps_b.tile([128, W], F32)
                        for kt in range(nkt):
                            ot = oht[:, (ktb + kt) * 128:(ktb + kt + 1) * 128]
                            first = kt == 0
                            last = kt == nkt - 1
                            nc.tensor.matmul(
                                px1[:, 0:512], ot, gt[:, ktb + kt, 0:512],
                                start=first, stop=last)
                            nc.tensor.matmul(
                                px1[:, 512:1024], ot,
                                gt[:, ktb + kt, 512:1024],
                                start=first, stop=last)
                        ktb += nkt
                        jsl = slice(j * W, (j + 1) * W)
                        if p == 0:
                            # fold z0 in at piece 0
                            nc.vector.tensor_add(x1acc[:, jsl], px1[:],
                                                 z0[:, jsl])
                        else:
                            nc.vector.tensor_add(x1acc[:, jsl],
                                                 x1acc[:, jsl], px1[:])
                        if p == AGS - 1:
                            # ---- phase C for block j ----
                            ct = pcl.tile([128, W], BF16, tag="c")
                            nc.scalar.activation(
                                ct[:], x1acc[:, jsl],
                                mybir.ActivationFunctionType.Tanh)
                            hxt = pcl.tile([128, W], BF16, tag="hx")
                            nc.vector.tensor_sub(hxt[:], hx_sb[:, j, :],
                                                 ct[:])
                            nc.vector.tensor_mul(hxt[:], hxt[:],
                                                 usb[:, jsl])
                            outb = pcl.tile([128, W], F32, tag="out")
                            nc.vector.tensor_add(outb[:], hxt[:], ct[:])
                            nc.sync.dma_start(
                                out_dram[j],
                                outb[:].rearrange("n (b u) -> n b u", b=B))
                    kt_off += nkt_h

    nc.compile()
    return nc


# ---------------- host side ----------------

def prep_inputs(inputs, hx, rows, cols, vals, W_fc, b_fc, W_g, b_g):
    """Build the 8 per-core input maps + the edge-group geometry."""
    import ml_dtypes
    F8NP = ml_dtypes.float8_e4m3
    xi = np.asarray(inputs).reshape(B, N, DIN)
    h = np.asarray(hx).reshape(B, N, U)
    rows = np.asarray(rows); cols = np.asarray(cols); vals = np.asarray(vals)

    core_of = rows // NPC
    piece_of = _src_piece(cols)
    per_core = []           # per core: dict[(p, j)] -> (cols, dest_local, val)
    counts = np.zeros((NCORES, AGS * NBLK), np.int64)
    for k in range(NCORES):
        m = core_of == k
        r_l = rows[m] - k * NPC
        c_l = cols[m]
        v_l = vals[m]
        p_l = piece_of[m]
        blk = r_l // 128
        groups = {}
        for gi, (p, j) in enumerate(GROUPS):
            gm = (blk == j) & (p_l == p)
            groups[(p, j)] = (c_l[gm], r_l[gm] % 128, v_l[gm])
            counts[k, gi] = gm.sum()
        per_core.append(groups)

    eb = tuple(int(-(-max(1, counts[:, gi].max()) // 128) * 128)
               for gi in range(AGS * NBLK))
    kt_tot = sum(e // 128 for e in eb)

    # feature order everywhere: [h (0:64), xi (64:66), ones (66)]
    perm = np.concatenate([np.arange(DIN, D), np.arange(DIN)])
    wfc_ext = np.concatenate(
        [np.asarray(W_fc)[perm], np.asarray(b_fc)[None, :]],
        axis=0).astype(np.float32)          # (67,128)
    wg = np.asarray(W_g).reshape(D, 2, U)
    wg_comb = np.zeros((K, 128), np.float32)
    wg_comb[:D, :U] = wg[perm, 1, :]       # odd rows -> Y
    wg_comb[:D, U:] = wg[perm, 0, :]       # even rows -> Z0
    wg_comb[D, U:] = np.asarray(b_g)       # b_g into Z0

    in_maps = []
    for k in range(NCORES):
        sl = slice(k * NPC, (k + 1) * NPC)
        xi_p = np.zeros((B, NPC_PAD, DIN), np.float32)
        xi_p[:, :NPC] = xi[:, sl]
        h_p = np.zeros((B, NPC_PAD, U), np.float32)
        h_p[:, :NPC] = h[:, sl]
        # rows ordered (blk, b, nl): tile t = blk*16 + b
        catT = np.empty((K, ROWS), np.float32)
        catT[0:U] = (h_p.reshape(B, NBLK, 128, U)
                     .transpose(3, 1, 0, 2).reshape(U, ROWS))
        catT[U:D] = (xi_p.reshape(B, NBLK, 128, DIN)
                     .transpose(3, 1, 0, 2).reshape(DIN, ROWS))
        catT[D] = 1.0
        hx_blk = (h_p.reshape(B, NBLK, 128, U)
                  .transpose(2, 1, 0, 3).reshape(128, NBLK, B * U))

        idx_w = np.zeros((128, sum(eb) // 16), np.int16)
        oh = np.zeros((128, kt_tot * 128), F8NP)
        ioff = 0
        ktoff = 0
        for gi, (p, j) in enumerate(GROUPS):
            c_l, dl, v_l = per_core[k][(p, j)]
            ne = len(c_l)
            idx = np.zeros(eb[gi], np.int16)
            idx[:ne] = _ytab_row(c_l // NPC, c_l % NPC).astype(np.int16)
            wrap = idx.reshape(eb[gi] // 16, 16).T        # (16, eb/16)
            idx_w[:, ioff: ioff + eb[gi] // 16] = np.tile(wrap, (8, 1))
            e_pos = np.arange(ne)
            oh[e_pos % 128, (ktoff + e_pos // 128) * 128 + dl] = \
                v_l.astype(np.float32)
            ioff += eb[gi] // 16
            ktoff += eb[gi] // 128

        in_maps.append({
            "catT": catT.astype(ml_dtypes.bfloat16),
            "hx_blk": hx_blk.astype(ml_dtypes.bfloat16),
            "wfc": wfc_ext.astype(ml_dtypes.bfloat16),
            "wg": wg_comb.astype(ml_dtypes.bfloat16),
            "idxw": idx_w,
            "oh": oh,
        })
    return eb, in_maps


_CACHE: dict = {}


def assemble_out(results):
    """results[k]['out'] is (NBLK, 128, B, U); -> (B, N*U)."""
    full = []
    for k in range(NCORES):
        o = results[k]["out"].reshape(NPC_PAD, B, U)[:NPC]   # (1250, B, U)
        full.append(o)
    o = np.concatenate(full, axis=0)                          # (N, B, U)
    return o.transpose(1, 0, 2).reshape(B, N * U)


def run(inputs, hx, rows, cols, vals, W_fc, b_fc, W_g, b_g):
    eb, in_maps = prep_inputs(inputs, hx, rows, cols, vals,
                              W_fc, b_fc, W_g, b_g)
    if eb not in _CACHE:
        _CACHE[eb] = build_kernel(eb)
    nc = _CACHE[eb]
    res = run_bass_kernel_spmd(nc, in_maps, core_ids=list(range(NCORES)))
    return assemble_out(res.results)


def kernel(inputs, hx, rows, cols, vals, W_fc, b_fc, W_g, b_g):
    """Harness entry: full (unsharded) inputs -> full output (B, N*U)."""
    out = run(inputs, hx, rows, cols, vals, W_fc, b_fc, W_g, b_g)
    return out.astype(np.float32)


# revision 6
# speedup vs baseline: 1.0903x; 1.0903x over previous
"""GCGRUCell Trainium2 kernel — 8-core SPMD (v3: fp8 gather path, piecewise
AG/BC pipelining, consolidated gathers).

Math (per reference):
  value = sigmoid(cat([x, h]) @ W_fc + b_fc);  r, u = split(value)
  X0 = cat([x, r*h])                                (B, N, D)   D=66
  Y  = X0 @ Wg_odd                                  (B, N, U)
  X1 = S @ Y          (segment_sum over E edges, commuted with Wg_odd)
  c  = tanh(X0 @ Wg_even + X1 + b_g)
  out = u * hx + (1-u) * c

Sharding: nodes 1250/core (phase A local, all batches); Y AllGather to a
(10240, 1024) fp8e4 table in 5 pieces; phase B edges are grouped by
(dest block, source AG piece) and processed piece-major with one gather
per (piece, half-of-blocks) — the gather for piece p waits only on AG
piece p, so phase B pipelines with the AllGather. Per-block
one-hot-vals fp8 matmuls on PE (fp32 PSUM) accumulate into an SBUF x1
accumulator (z0 folded in at piece 0); phase C combines and writes the
core's output slice.

Row order: tile t in [0,160) = (blk = t//16, b = t%16), 128 nodes per
tile. Feature order: [h (0:64), xi (64:66), ones (66)] (32-aligned
partition bases). Biases ride the ones row.
"""

from contextlib import nullcontext

import numpy as np
import concourse.bass as bass
import concourse.bacc as bacc
import concourse.mybir as mybir
import concourse.tile as tile
from bass_rust import add_dep_helper
from concourse.bass_utils import run_bass_kernel_spmd

F32 = mybir.dt.float32
BF16 = mybir.dt.bfloat16
F8 = mybir.dt.float8e4
I16 = mybir.dt.int16

NCORES = 8
N, B, U, DIN = 10000, 16, 64, 2
D = DIN + U                      # 66
K = D + 1                        # 67 (ones row for biases)
NPC = N // NCORES                # 1250 nodes/core
NPC_PAD = 1280                   # 10 blocks of 128
NBLK = NPC_PAD // 128            # 10 dest blocks
ROWS = B * NPC_PAD               # 20480
CHUNK_ROWS = 16 * 128            # one block of nodes x all batches
GPC = CHUNK_ROWS // 512          # 4 512-col groups per chunk
W = B * U                        # 1024 gather row width
AGS = 5                          # allgather split count
AG_BOUNDS = [round(i * NBLK / AGS) for i in range(AGS + 1)]
GROUPS = [(p, j) for p in range(AGS) for j in range(NBLK)]
HALVES = [(p, h) for p in range(AGS) for h in range(2)]
JPH = NBLK // 2                  # dest blocks per gather call
NQ = 1                           # DMA queues to spread gathers over


def _ytab_row(c, nl):
    """y_tab row index of node (core c, local node nl) after the split
    AllGather: piece p's output is [rank0 blocks j0:j1, rank1 blocks
    j0:j1, ...] concatenated."""
    j = nl // 128
    si = nl % 128
    bounds = np.asarray(AG_BOUNDS)
    p = np.searchsorted(bounds, j, side="right") - 1
    j0 = bounds[p]
    j1 = bounds[p + 1]
    return (j0 * NCORES + c * (j1 - j0) + (j - j0)) * 128 + si


def _src_piece(c):
    """AG piece that carries the y_tab row of global node c."""
    j = (c % NPC) // 128
    return np.searchsorted(np.asarray(AG_BOUNDS), j, side="right") - 1


def build_kernel(eb: tuple[int, ...], stage: int = 5,
                 r_a: int = 1, r_ag: int = 1, r_bc: int = 1):
    """eb[g] = padded edge count of group g = (piece-major) (p, j),
    multiple of 128 (same for all cores).
    stage: 1=phase A only, 2=+allgather, 3=+gathers, 4=full
    r_a/r_ag/r_bc: repetition counts (hardware For_i loops / replicated
    collective) for wall-clock-difference timing; correctness needs 1."""
    assert len(eb) == AGS * NBLK and all(e % 128 == 0 for e in eb)
    kt_tot = sum(e // 128 for e in eb)
    # per (piece, half): total padded edges of its JPH dest-block groups
    eh = [sum(eb[p * NBLK + h * JPH + j] for j in range(JPH))
          for (p, h) in HALVES]
    ktmax = max(e // 128 for e in eh)

    nc = bacc.Bacc("TRN2", target_bir_lowering=False, debug=False,
                   num_devices=NCORES)

    # ---- I/O ----
    catT_in = nc.dram_tensor("catT", [K, ROWS], BF16, kind="ExternalInput")
    hxb_in = nc.dram_tensor("hx_blk", [128, NBLK, W], BF16,
                            kind="ExternalInput")
    wfc_in = nc.dram_tensor("wfc", [K, 128], BF16, kind="ExternalInput")
    wg_in = nc.dram_tensor("wg", [K, 128], BF16, kind="ExternalInput")
    idx_in = nc.dram_tensor("idxw", [128, sum(eb) // 16], I16,
                            kind="ExternalInput")
    oh_in = nc.dram_tensor("oh", [128, kt_tot * 128], F8,
                           kind="ExternalInput")
    out_dram = nc.dram_tensor("out", [NBLK, 128, B, U], F32,
                              kind="ExternalOutput")

    # ---- internal DRAM (collective) ----
    y_loc = nc.dram_tensor("y_loc", [NBLK * 128, W], F8, kind="Internal")
    y_tab = nc.dram_tensor("y_tab", [NBLK * NCORES * 128, W], F8,
                           kind="Internal", addr_space="Shared")

    with tile.TileContext(nc) as tc:
        with (
            tc.tile_pool(name="persist", bufs=1) as pp,
        ):
            wfc = pp.tile([K, 128], BF16)
            nc.sync.dma_start(wfc[:], wfc_in[:])
            wg = pp.tile([K, 128], BF16)
            nc.sync.dma_start(wg[:], wg_in[:])
            z0 = pp.tile([128, NBLK * W], BF16)     # 20KB/part
            usb = pp.tile([128, NBLK * W], BF16)    # 20KB/part
            x1acc = pp.tile([128, NBLK * W], F32)   # 40KB/part
            hx_sb = pp.tile([128, NBLK, W], BF16)   # 20KB/part

            # ================= PHASE A =================
            y_writes = []
            with (
                tc.tile_pool(name="pa", bufs=2) as pa,
                tc.tile_pool(name="pa_sig", bufs=3) as psig,
                tc.tile_pool(name="pa_y", bufs=2) as py,
                tc.tile_pool(name="ps_r", bufs=2, space="PSUM") as ps_r,
                tc.tile_pool(name="ps_u", bufs=2, space="PSUM") as ps_u,
                tc.tile_pool(name="ps_yz", bufs=2, space="PSUM") as ps_yz,
                tc.For_i(0, r_a, 1) if r_a > 1 else nullcontext(),
            ):
                for blk in range(NBLK):
                    catT = pa.tile([K, CHUNK_ROWS], BF16, tag="catT")
                    nc.sync.dma_start(
                        catT[:],
                        catT_in[:, blk * CHUNK_ROWS:(blk + 1) * CHUNK_ROWS])
                    x0t = pa.tile([K, CHUNK_ROWS], BF16, tag="x0t")
                    nc.vector.tensor_copy(x0t[U:K, :], catT[U:K, :])
                    for g in range(GPC):
                        sl = slice(g * 512, (g + 1) * 512)
                        pr = ps_r.tile([U, 512], F32)
                        nc.tensor.matmul(pr[:], wfc[:, 0:U], catT[:, sl],
                                         start=True, stop=True)
                        sig = psig.tile([U, 512], BF16)
                        nc.scalar.activation(
                            sig[:], pr[:], mybir.ActivationFunctionType.Sigmoid)
                        nc.vector.tensor_mul(
                            x0t[0:U, sl], sig[:], catT[0:U, sl])
                    ystage = py.tile([128, W], F8, tag="ystage")
                    # u-logits: 8 tiles share one PSUM bank -> 1 big sigmoid
                    for half in range(2):
                        pu = ps_u.tile([128, 512], F32)
                        for i in range(8):
                            b = half * 8 + i
                            tsl = slice(b * 128, (b + 1) * 128)
                            nc.tensor.matmul(
                                pu[:, i * U:(i + 1) * U],
                                catT[:, tsl], wfc[:, U:128],
                                start=True, stop=True)
                        nc.scalar.activation(
                            usb[:, blk * W + half * 512:
                                 blk * W + (half + 1) * 512],
                            pu[:], mybir.ActivationFunctionType.Sigmoid)
                    # Y and Z0' as separate families: 8 tiles share a PSUM
                    # bank each -> 2 big contiguous copies per half
                    for half in range(2):
                        pyy = ps_yz.tile([128, 512], F32, tag="yy")
                        pzz = ps_yz.tile([128, 512], F32, tag="zz")
                        for i in range(8):
                            b = half * 8 + i
                            tsl = slice(b * 128, (b + 1) * 128)
                            nc.tensor.matmul(
                                pyy[:, i * U:(i + 1) * U],
                                x0t[:, tsl], wg[:, 0:U],
                                start=True, stop=True)
                            nc.tensor.matmul(
                                pzz[:, i * U:(i + 1) * U],
                                x0t[:, tsl], wg[:, U:128],
                                start=True, stop=True)
                        nc.vector.tensor_copy(
                            ystage[:, half * 512:(half + 1) * 512], pyy[:])
                        nc.vector.tensor_copy(
                            z0[:, blk * W + half * 512:
                               blk * W + (half + 1) * 512], pzz[:])
                    ydma = nc.sync.dma_start(
                        y_loc[blk * 128:(blk + 1) * 128, :], ystage[:])
                    y_writes.append(ydma)

            # hx prefetch for phase C (after phase A's catT loads in the
            # sync queue so it doesn't delay them)
            nc.sync.dma_start(hx_sb[:], hxb_in[:])

            # ================= ALLGATHER (split, overlaps phase A) ======
            ccs = []
            if stage >= 2:
                prev_cc = None
                for rep in range(r_ag):
                    for p in range(AGS):
                        j0, j1 = AG_BOUNDS[p], AG_BOUNDS[p + 1]
                        cc = nc.gpsimd.collective_compute(
                            "AllGather", mybir.AluOpType.bypass,
                            replica_groups=[list(range(NCORES))],
                            ins=[y_loc[j0 * 128: j1 * 128, :]],
                            outs=[y_tab[j0 * NCORES * 128:
                                        j1 * NCORES * 128, :]],
                        )
                        ccs.append(cc)
                        if r_ag > 1 and prev_cc is not None:
                            add_dep_helper(cc.ins, prev_cc.ins, sync=True,
                                           reason="serialize ag reps")
                        prev_cc = cc
                        if r_a == 1 and r_ag == 1:
                            for j in range(j0, j1):
                                add_dep_helper(cc.ins, y_writes[j].ins,
                                               sync=True,
                                               reason="allgather reads y_loc")

            # ================= PHASE B + C =================
            with (
                tc.tile_pool(name="pg", bufs=3) as pg,
                tc.tile_pool(name="pi", bufs=3) as pi,
                tc.tile_pool(name="po", bufs=2) as po,
                tc.tile_pool(name="pc", bufs=2) as pcl,
                tc.tile_pool(name="ps_b", bufs=2, space="PSUM") as ps_b,
                tc.For_i(0, r_bc, 1) if r_bc > 1 else nullcontext(),
            ):
                idx_off = 0   # in int16 columns (16 idx each)
                kt_off = 0
                for hi, (p, h) in enumerate(HALVES if stage >= 3 else []):
                    e_h = eh[hi]
                    nkt_h = e_h // 128
                    gt = pg.tile([128, ktmax, W], F8, tag="G")
                    if stage != 6:   # stage 6: skip the gather itself
                        idxt = pi.tile([128, max(eh) // 16], I16, tag="idx")
                        nc.sync.dma_start(
                            idxt[:, :e_h // 16],
                            idx_in[:, idx_off: idx_off + e_h // 16])
                        gather = nc.gpsimd.dma_gather(
                            out_ap=gt[:, :nkt_h, :],
                            in_ap=y_tab[:],
                            idxs_ap=idxt[:, :e_h // 16],
                            num_idxs=e_h,
                            num_idxs_reg=e_h,
                            elem_size=W,
                            single_packet=False,
                            queue_num=hi % NQ,
                        )
                        if r_bc == 1 and r_ag == 1:
                            add_dep_helper(
                                gather.ins, ccs[p].ins, sync=True,
                                reason="gather reads allgathered y_tab piece")
                    idx_off += e_h // 16
                    if stage < 4:
                        kt_off += nkt_h
                        continue
                    oht = po.tile([128, ktmax * 128], F8, tag="O")
                    nc.sync.dma_start(
                        oht[:, :nkt_h * 128],
                        oh_in[:, kt_off * 128:(kt_off + nkt_h) * 128])
                    ktb = 0
                    for jj in range(JPH):
                        j = h * JPH + jj
                        nkt = eb[p * NBLK + j] // 128
                        px1 = ps_b.tile([128, W], F32)
                        for kt in range(nkt):
                            ot = oht[:, (ktb + kt) * 128:(ktb + kt + 1) * 128]
                            first = kt == 0
                            last = kt == nkt - 1
                            nc.tensor.matmul(
                                px1[:, 0:512], ot, gt[:, ktb + kt, 0:512],
                                start=first, stop=last)
                            nc.tensor.matmul(
                                px1[:, 512:1024], ot,
                                gt[:, ktb + kt, 512:1024],
                                start=first, stop=last)
                        ktb += nkt
                        jsl = slice(j * W, (j + 1) * W)
                        if p == 0:
                            # fold z0 in at piece 0
                            nc.vector.tensor_add(x1acc[:, jsl], px1[:],
                                                 z0[:, jsl])
                        else:
                            nc.vector.tensor_add(x1acc[:, jsl],
                                                 x1acc[:, jsl], px1[:])
                        if p == AGS - 1:
                            # ---- phase C for block j ----
                            ct = pcl.tile([128, W], BF16, tag="c")
                            nc.scalar.activation(
                                ct[:], x1acc[:, jsl],
                                mybir.ActivationFunctionType.Tanh)
                            hxt = pcl.tile([128, W], BF16, tag="hx")
                            nc.vector.tensor_sub(hxt[:], hx_sb[:, j, :],
                                                 ct[:])
                            nc.vector.tensor_mul(hxt[:], hxt[:],
                                                 usb[:, jsl])
                            outb = pcl.tile([128, W], F32, tag="out")
                            nc.vector.tensor_add(outb[:], hxt[:], ct[:])
                            nc.sync.dma_start(
                                out_dram[j],
                                outb[:].rearrange("n (b u) -> n b u", b=B))
                    kt_off += nkt_h

    nc.compile()
    return nc


# ---------------- host side ----------------

def prep_inputs(inputs, hx, rows, cols, vals, W_fc, b_fc, W_g, b_g):
    """Build the 8 per-core input maps + the edge-group geometry."""
    import ml_dtypes
    F8NP = ml_dtypes.float8_e4m3
    xi = np.asarray(inputs).reshape(B, N, DIN)
    h = np.asarray(hx).reshape(B, N, U)
    rows = np.asarray(rows); cols = np.asarray(cols); vals = np.asarray(vals)

    core_of = rows // NPC
    piece_of = _src_piece(cols)
    per_core = []           # per core: dict[(p, j)] -> (cols, dest_local, val)
    counts = np.zeros((NCORES, AGS * NBLK), np.int64)
    for k in range(NCORES):
        m = core_of == k
        r_l = rows[m] - k * NPC
        c_l = cols[m]
        v_l = vals[m]
        p_l = piece_of[m]
        blk = r_l // 128
        groups = {}
        for gi, (p, j) in enumerate(GROUPS):
            gm = (blk == j) & (p_l == p)
            groups[(p, j)] = (c_l[gm], r_l[gm] % 128, v_l[gm])
            counts[k, gi] = gm.sum()
        per_core.append(groups)

    eb = tuple(int(-(-max(1, counts[:, gi].max()) // 128) * 128)
               for gi in range(AGS * NBLK))
    kt_tot = sum(e // 128 for e in eb)

    # feature order everywhere: [h (0:64), xi (64:66), ones (66)]
    perm = np.concatenate([np.arange(DIN, D), np.arange(DIN)])
    wfc_ext = np.concatenate(
        [np.asarray(W_fc)[perm], np.asarray(b_fc)[None, :]],
        axis=0).astype(np.float32)          # (67,128)
    wg = np.asarray(W_g).reshape(D, 2, U)
    wg_comb = np.zeros((K, 128), np.float32)
    wg_comb[:D, :U] = wg[perm, 1, :]       # odd rows -> Y
    wg_comb[:D, U:] = wg[perm, 0, :]       # even rows -> Z0
    wg_comb[D, U:] = np.asarray(b_g)       # b_g into Z0

    in_maps = []
    for k in range(NCORES):
        sl = slice(k * NPC, (k + 1) * NPC)
        xi_p = np.zeros((B, NPC_PAD, DIN), np.float32)
        xi_p[:, :NPC] = xi[:, sl]
        h_p = np.zeros((B, NPC_PAD, U), np.float32)
        h_p[:, :NPC] = h[:, sl]
        # rows ordered (blk, b, nl): tile t = blk*16 + b
        catT = np.empty((K, ROWS), np.float32)
        catT[0:U] = (h_p.reshape(B, NBLK, 128, U)
                     .transpose(3, 1, 0, 2).reshape(U, ROWS))
        catT[U:D] = (xi_p.reshape(B, NBLK, 128, DIN)
                     .transpose(3, 1, 0, 2).reshape(DIN, ROWS))
        catT[D] = 1.0
        hx_blk = (h_p.reshape(B, NBLK, 128, U)
                  .transpose(2, 1, 0, 3).reshape(128, NBLK, B * U))

        idx_w = np.zeros((128, sum(eb) // 16), np.int16)
        oh = np.zeros((128, kt_tot * 128), F8NP)
        ioff = 0
        ktoff = 0
        for gi, (p, j) in enumerate(GROUPS):
            c_l, dl, v_l = per_core[k][(p, j)]
            ne = len(c_l)
            idx = np.zeros(eb[gi], np.int16)
            idx[:ne] = _ytab_row(c_l // NPC, c_l % NPC).astype(np.int16)
            wrap = idx.reshape(eb[gi] // 16, 16).T        # (16, eb/16)
            idx_w[:, ioff: ioff + eb[gi] // 16] = np.tile(wrap, (8, 1))
            e_pos = np.arange(ne)
            oh[e_pos % 128, (ktoff + e_pos // 128) * 128 + dl] = \
                v_l.astype(np.float32)
            ioff += eb[gi] // 16
            ktoff += eb[gi] // 128

        in_maps.append({
            "catT": catT.astype(ml_dtypes.bfloat16),
            "hx_blk": hx_blk.astype(ml_dtypes.bfloat16),
            "wfc": wfc_ext.astype(ml_dtypes.bfloat16),
            "wg": wg_comb.astype(ml_dtypes.bfloat16),
            "idxw": idx_w,
            "oh": oh,
        })
    return eb, in_maps


_CACHE: dict = {}


def assemble_out(results):
    """results[k]['out'] is (NBLK, 128, B, U); -> (B, N*U)."""
    full = []
    for k in range(NCORES):
        o = results[k]["out"].reshape(NPC_PAD, B, U)[:NPC]   # (1250, B, U)
        full.append(o)
    o = np.concatenate(full, axis=0)                          # (N, B, U)
    return o.transpose(1, 0, 2).reshape(B, N * U)


def run(inputs, hx, rows, cols, vals, W_fc, b_fc, W_g, b_g):
    eb, in_maps = prep_inputs(inputs, hx, rows, cols, vals,
                              W_fc, b_fc, W_g, b_g)
    if eb not in _CACHE:
        _CACHE[eb] = build_kernel(eb)
    nc = _CACHE[eb]
    res = run_bass_kernel_spmd(nc, in_maps, core_ids=list(range(NCORES)))
    return assemble_out(res.results)


def kernel(inputs, hx, rows, cols, vals, W_fc, b_fc, W_g, b_g):
    """Harness entry: full (unsharded) inputs -> full output (B, N*U)."""
    out = run(inputs, hx, rows, cols, vals, W_fc, b_fc, W_g, b_g)
    return out.astype(np.float32)
